# revision 3
# baseline (speedup 1.0000x reference)
"""GPT forward pass on 8 Trainium2 NeuronCores — collective-minimal design.

Sharding:
  - Residual stream x kept TRANSPOSED [D, T_local], sequence-sharded
    (T=2048 tokens -> 256 per core), SBUF-resident.
  - Attention: tensor-parallel over heads (2 heads/core). AllGather the
    LN1 output over tokens (8-rank, 4MB out), compute Q/K feature-major +
    V token-major, causal softmax without max-subtraction, AllToAll back
    to sequence sharding (1MB). 2 collectives per layer.
  - MLP: fully token-local — each core does its own 256 tokens x full
    DFF=4096, streaming the full up/down weights (16MB/layer bf16)
    overlapped with compute. Zero collectives.
  - Logits: token-local over the full 32000 vocab, streaming out_w
    (64MB bf16). Zero collectives; output is naturally token-sharded.
"""

import numpy as np

import concourse.bass as bass
import concourse.mybir as mybir
import concourse.tile as tile
from concourse import bacc
from concourse.bass_utils import run_bass_kernel_spmd
from concourse.masks import make_identity

NCORES = 8
V, L, D, NB, H = 32000, 1024, 1024, 8, 16
HD = D // H  # 64
DFF = 4 * D
B, S = 2, 1024
T = B * S
TL = T // NCORES  # 256 local tokens
DC = D // 128  # 8 feature chunks
G = 4  # ranks per batch group
HL = H // G  # 4 local heads
NPAIR = HL // 2  # head pairs per core
VW = 500  # vocab tile width
NVT = V // VW  # 64
EPS = 1e-5

F32 = mybir.dt.float32
F32R = mybir.dt.float32r
BF16 = mybir.dt.bfloat16
I32 = mybir.dt.int32
AF = mybir.ActivationFunctionType
OP = mybir.AluOpType
RG = [list(range(NCORES))]
RG2 = [[0, 1, 2, 3], [4, 5, 6, 7]]

CT = BF16  # comm/activation dtype
WT = BF16  # weight dtype

_CACHE: dict = {}


def _build(iters: int = 1):
    nc = bacc.Bacc("TRN2", num_devices=NCORES)

    # ---------------- inputs (per-core shards, host-prepped) ----------------
    d_idx = nc.dram_tensor("idx", [TL, 1], I32, kind="ExternalInput")
    d_tok = nc.dram_tensor("tok", [V, D], F32, kind="ExternalInput")
    d_pos = nc.dram_tensor("pos", [2, 128, D], F32, kind="ExternalInput")
    d_wqkv = nc.dram_tensor("wqkv", [NB, D, 3 * HL * HD], WT, kind="ExternalInput")
    d_upw = nc.dram_tensor("upw", [NB, 8, 128, DC, 512], WT, kind="ExternalInput")
    d_upb = nc.dram_tensor("upb", [NB, 128, 32], F32, kind="ExternalInput")
    d_dww = nc.dram_tensor("dww", [NB, 2, 32, 128, 4, 128], WT, kind="ExternalInput")
    d_dwb = nc.dram_tensor("dwb", [NB, 128, 8], F32, kind="ExternalInput")
    d_ln1w = nc.dram_tensor("ln1w", [NB, 128, DC], F32, kind="ExternalInput")
    d_ln1b = nc.dram_tensor("ln1b", [NB, 128, DC], F32, kind="ExternalInput")
    d_ln2w = nc.dram_tensor("ln2w", [NB, 128, DC], F32, kind="ExternalInput")
    d_ln2b = nc.dram_tensor("ln2b", [NB, 128, DC], F32, kind="ExternalInput")
    d_lnfw = nc.dram_tensor("lnfw", [128, DC], F32, kind="ExternalInput")
    d_lnfb = nc.dram_tensor("lnfb", [128, DC], F32, kind="ExternalInput")
    d_outw = nc.dram_tensor("outw", [NVT, 128, DC, VW], WT, kind="ExternalInput")
    d_outb = nc.dram_tensor("outb", [1, V], F32R, kind="ExternalInput")
    d_ones = nc.dram_tensor("ones", [128, 1024], F32R, kind="ExternalInput")
    d_onesb = nc.dram_tensor("onesb", [128, 32], CT, kind="ExternalInput")
    d_mask = nc.dram_tensor("mask", [128, 4, 512], CT, kind="ExternalInput")
    d_a2aidx = nc.dram_tensor("a2aidx", [128, DC], I32, kind="ExternalInput")

    d_logits = nc.dram_tensor("logits", [TL, V], F32, kind="ExternalOutput")

    from contextlib import ExitStack

    with nc.allow_low_precision(reason="bf16/f32r intended throughout"), tile.TileContext(nc) as tc:
        with ExitStack() as stack:
            ep = stack.enter_context
            constp = ep(tc.tile_pool(name="const", bufs=1))
            xp = ep(tc.tile_pool(name="xp", bufs=1))
            lnp = ep(tc.tile_pool(name="lnp", bufs=1))
            wp = ep(tc.tile_pool(name="wp", bufs=1))
            wsp = ep(tc.tile_pool(name="wsp", bufs=4))
            ap = ep(tc.tile_pool(name="ap", bufs=2))
            qkvp = ep(tc.tile_pool(name="qkv", bufs=1))
            hpool = ep(tc.tile_pool(name="hp", bufs=1))
            rowp = ep(tc.tile_pool(name="row", bufs=1))
            psmm = ep(tc.tile_pool(name="psmm", bufs=2, space="PSUM"))
            pspv = ep(tc.tile_pool(name="pspv", bufs=1, space="PSUM"))
            psbc = ep(tc.tile_pool(name="psbc", bufs=1, space="PSUM"))
            psd = ep(tc.tile_pool(name="psd", bufs=4, space="PSUM"))
            dramp = ep(tc.tile_pool(name="dram", bufs=1, space="DRAM"))
            del ep
            ones = constp.tile([128, 1024], F32R)
            nc.sync.dma_start(ones[:], d_ones[:])
            ident = constp.tile([128, 128], F32)
            make_identity(nc, ident)
            masks = constp.tile([128, 4, 512], CT)
            nc.sync.dma_start(masks[:], d_mask[:])
            lnfw_t = constp.tile([128, DC], F32)
            nc.sync.dma_start(lnfw_t[:], d_lnfw[:])
            lnfb_t = constp.tile([128, DC], F32)
            nc.sync.dma_start(lnfb_t[:], d_lnfb[:])

            ones_col = ones[:, 0:1]  # [128,1] lhsT for partition sums

            for _it in range(iters):
                x = xp.tile([128, DC, TL], F32R, name="x_resid")
                a2aidx_t = rowp.tile([128, DC], I32, tag="a2aidx")
                nc.sync.dma_start(a2aidx_t[:], d_a2aidx[:])

                # ---------------- embedding ----------------
                emb = ap.tile([128, 2, D], F32, tag="emb")
                idx_t = rowp.tile([128, 2], I32, tag="idx")
                nc.sync.dma_start(idx_t[:], d_idx.rearrange("(c p) o -> p (c o)", p=128))
                for c in range(2):
                    nc.gpsimd.indirect_dma_start(
                        emb[:, c, :],
                        None,
                        d_tok[:],
                        bass.IndirectOffsetOnAxis(ap=idx_t[:, c : c + 1], axis=0),
                    )
                    for hf in range(2):
                        pos_t = ap.tile([128, 512], F32, tag="pos")
                        nc.sync.dma_start(pos_t[:], d_pos[c, :, 512 * hf : 512 * (hf + 1)])
                        nc.vector.tensor_add(
                            emb[:, c, 512 * hf : 512 * (hf + 1)],
                            emb[:, c, 512 * hf : 512 * (hf + 1)],
                            pos_t[:],
                        )
                for c in range(2):
                    for dc in range(DC):
                        tp = psd.tile([128, 512], F32, tag="psd")
                        nc.tensor.transpose(
                            tp[:, 0:128], emb[:, c, dc * 128 : (dc + 1) * 128], ident[:]
                        )
                        nc.vector.tensor_copy(
                            x[:, dc, c * 128 : (c + 1) * 128], tp[:, 0:128]
                        )

                # ---------------- helpers ----------------
                def layernorm(dst, w_t, b_t):
                    """LN over feature axis of x [128, DC, TL] -> dst SBUF [128, DC, TL] CT."""
                    ps_st = psbc.tile([1, 2 * TL], F32, tag="bc")
                    for dc in range(DC):
                        nc.tensor.matmul(
                            ps_st[:, 0:TL], ones_col, x[:, dc, :],
                            start=(dc == 0), stop=(dc == DC - 1),
                        )
                    for dc in range(DC):
                        sq = ap.tile([128, TL], F32R, tag="lnsq")
                        nc.vector.tensor_mul(sq[:], x[:, dc, :], x[:, dc, :])
                        nc.tensor.matmul(
                            ps_st[:, TL : 2 * TL], ones_col, sq[:],
                            start=(dc == 0), stop=(dc == DC - 1),
                        )
                    mu = rowp.tile([1, TL], F32, tag="mu")
                    nc.vector.tensor_scalar(
                        out=mu[:], in0=ps_st[:, 0:TL], scalar1=1.0 / D, scalar2=None,
                        op0=OP.mult,
                    )
                    var = rowp.tile([1, TL], F32, tag="var")
                    nc.vector.tensor_scalar(
                        out=var[:], in0=ps_st[:, TL : 2 * TL], scalar1=1.0 / D,
                        scalar2=EPS, op0=OP.mult, op1=OP.add,
                    )
                    mu2 = rowp.tile([1, TL], F32, tag="mu2")
                    nc.vector.tensor_mul(mu2[:], mu[:], mu[:])
                    nc.vector.tensor_sub(var[:], var[:], mu2[:])
                    row = rowp.tile([1, 2 * TL], F32R, tag="row")
                    rootv = rowp.tile([1, TL], F32, tag="rootv")
                    nc.scalar.activation(rootv[:], var[:], AF.Sqrt)
                    nc.vector.reciprocal(row[:, 0:TL], rootv[:])
                    negmu = rowp.tile([1, TL], F32, tag="negmu")
                    nc.vector.tensor_scalar(
                        out=negmu[:], in0=mu[:], scalar1=-1.0, scalar2=None, op0=OP.mult,
                    )
                    nc.vector.tensor_mul(row[:, TL : 2 * TL], negmu[:], row[:, 0:TL])
                    ps_bc = psbc.tile([128, 512], F32, tag="bc")
                    nc.tensor.matmul(
                        ps_bc[:, 0 : 2 * TL], ones[0:1, 0:128], row[:],
                        start=True, stop=True,
                    )
                    for dc in range(DC):
                        t1 = ap.tile([128, TL], F32, tag="ln_t1")
                        nc.vector.tensor_mul(
                            t1[:], x[:, dc, :].bitcast(F32), ps_bc[:, 0:TL]
                        )
                        nc.vector.tensor_add(t1[:], t1[:], ps_bc[:, TL : 2 * TL])
                        nc.scalar.activation(
                            dst[:, dc, :], t1[:], AF.Identity,
                            bias=b_t[:, dc : dc + 1], scale=w_t[:, dc : dc + 1],
                        )

                def ki_list(qj):
                    out = []
                    for ki in range(8):
                        if 128 * ki + 127 <= 512 * qj:
                            out.append((ki, None))  # fully below diagonal
                        elif 128 * ki <= 512 * qj + 511:
                            out.append((ki, ki - 4 * qj))  # diagonal tile
                    return out

                # ---------------- layers ----------------
                for li in range(NB):
                    # --- attention: LN1 -> AllGather -> QKV -> attn -> A2A ---
                    ln1w_t = wp.tile([128, DC], F32, tag="ln1w")
                    nc.sync.dma_start(ln1w_t[:], d_ln1w[li])
                    ln1b_t = wp.tile([128, DC], F32, tag="ln1b")
                    nc.sync.dma_start(ln1b_t[:], d_ln1b[li])
                    ln1sb = lnp.tile([128, DC, TL], CT, tag="lnout")
                    layernorm(ln1sb, ln1w_t, ln1b_t)
                    ag1_in = dramp.tile([D, TL], CT, tag="ag_in", bufs=2)
                    nc.sync.dma_start(
                        ag1_in.rearrange("(c p) t -> p c t", p=128), ln1sb[:]
                    )
                    ag1_out = dramp.tile([G * D, TL], CT, tag="ag_out", bufs=2)
                    nc.gpsimd.collective_compute(
                        "AllGather", OP.bypass, replica_groups=RG2,
                        ins=[ag1_in[:]], outs=[ag1_out[:]],
                    )
                    agv = ag1_out.rearrange("(j c p) t -> c p j t", j=G, p=128)

                    wqkv_t = wp.tile([128, DC, 3 * HL * HD], WT, tag="wqkv")
                    nc.sync.dma_start(
                        wqkv_t[:], d_wqkv[li].rearrange("(c p) m -> p c m", p=128)
                    )
                    a2a_in = dramp.tile(
                        [NCORES * HL * HD, TL], CT, tag="a2a_in", bufs=2
                    )
                    a2a_in_v = a2a_in.rearrange(
                        "(j h p) t -> p h j t", j=NCORES, h=HL, p=HD
                    )

                    q_T = qkvp.tile([128, NPAIR, S], CT, tag="qT")
                    k_T = qkvp.tile([128, NPAIR, S], CT, tag="kT")
                    v_n = qkvp.tile([128, NPAIR, 8, 130], CT, tag="vn")
                    for p in range(NPAIR):
                        nc.sync.dma_start(
                            v_n[:, p, :, 64:65], d_onesb[:, 0:8].unsqueeze(2)
                        )
                        nc.sync.dma_start(
                            v_n[:, p, :, 129:130], d_onesb[:, 8:16].unsqueeze(2)
                        )
                    for tt in range(2):  # 512-token spans of my batch
                        j0 = 2 * tt
                        xr = ap.tile([128, DC, 512], CT, tag="xrhs")
                        for dc in range(DC):
                            nc.sync.dma_start(xr[:, dc, :], agv[dc, :, j0 : j0 + 2, :])
                        for p in range(NPAIR):
                            ps_q = psmm.tile([128, 512], F32, tag="mm")
                            ps_k = psmm.tile([128, 512], F32, tag="mm")
                            for dc in range(DC):
                                nc.tensor.matmul(
                                    ps_q[:], wqkv_t[:, dc, 128 * p : 128 * (p + 1)],
                                    xr[:, dc, :],
                                    start=(dc == 0), stop=(dc == DC - 1),
                                )
                            for dc in range(DC):
                                nc.tensor.matmul(
                                    ps_k[:],
                                    wqkv_t[:, dc, 256 + 128 * p : 256 + 128 * (p + 1)],
                                    xr[:, dc, :],
                                    start=(dc == 0), stop=(dc == DC - 1),
                                )
                            nc.vector.tensor_copy(
                                q_T[:, p, 512 * tt : 512 * (tt + 1)], ps_q[:]
                            )
                            nc.vector.tensor_copy(
                                k_T[:, p, 512 * tt : 512 * (tt + 1)], ps_k[:]
                            )
                        for mt in range(4):
                            ps_vt = psd.tile([128, 512], F32, tag="psd")
                            for dc in range(DC):
                                nc.tensor.matmul(
                                    ps_vt[:, 0 : HL * HD],
                                    xr[:, dc, mt * 128 : (mt + 1) * 128],
                                    wqkv_t[:, dc, 512 : 512 + HL * HD],
                                    start=(dc == 0), stop=(dc == DC - 1),
                                )
                            ki = tt * 4 + mt
                            for p in range(NPAIR):
                                nc.scalar.activation(
                                    v_n[:, p, ki, 0:64],
                                    ps_vt[:, 128 * p : 128 * p + 64], AF.Copy,
                                )
                                nc.scalar.activation(
                                    v_n[:, p, ki, 65:129],
                                    ps_vt[:, 128 * p + 64 : 128 * p + 128], AF.Copy,
                                )

                    def attend_head(p, hh, qj):
                        hp = 64 * hh
                        h_idx = 2 * p + hh
                        vc0 = 65 * hh
                        q_sl = q_T[hp : hp + 64, p, 512 * qj : 512 * (qj + 1)]
                        ps_pv = pspv.tile([65, 512], F32, tag="pv")
                        kis = ki_list(qj)
                        for en, (ki, mj) in enumerate(kis):
                            ps_s = psmm.tile([128, 512], F32, tag="mm")
                            nc.tensor.matmul(
                                ps_s[:],
                                k_T[hp : hp + 64, p, 128 * ki : 128 * (ki + 1)],
                                q_sl,
                                start=True, stop=True,
                            )
                            ex = ap.tile([128, 512], CT, tag="exp")
                            nc.scalar.activation(ex[:], ps_s[:], AF.Exp, scale=0.125)
                            if mj is not None:
                                nc.vector.tensor_mul(ex[:], ex[:], masks[:, mj, :])
                            nc.tensor.matmul(
                                ps_pv[:],
                                v_n[:, p, ki, vc0 : vc0 + 65],
                                ex[:],
                                start=(en == 0), stop=(en == len(kis) - 1),
                            )
                        recip = rowp.tile([1, 512], F32R, tag="recip")
                        nc.vector.reciprocal(recip[:], ps_pv[64:65, :])
                        ps_r = psbc.tile([128, 512], F32, tag="bc")
                        nc.tensor.matmul(
                            ps_r[0:64, :], ones[0:1, 0:64], recip[:],
                            start=True, stop=True,
                        )
                        au = ap.tile([64, 512], F32, tag="au")
                        nc.scalar.activation(au[:], ps_pv[0:64, :], AF.Copy)
                        asl = ap.tile([64, 512], CT, tag="asl")
                        nc.vector.tensor_mul(asl[:], au[:], ps_r[0:64, :])
                        j0 = 2 * qj
                        nc.sync.dma_start(
                            a2a_in_v[:, h_idx, j0 : j0 + 2, :], asl[:]
                        )
                        nc.sync.dma_start(
                            a2a_in_v[:, h_idx, 4 + j0 : 4 + j0 + 2, :], asl[:]
                        )

                    for p in range(NPAIR):
                        for hh in range(2):
                            for qj in range(2):
                                attend_head(p, hh, qj)

                    a2a_out = dramp.tile(
                        [NCORES * HL * HD, TL], CT, tag="a2a_out", bufs=2
                    )
                    nc.gpsimd.collective_compute(
                        "AllToAll", OP.bypass, replica_groups=RG,
                        ins=[a2a_in[:]], outs=[a2a_out[:]],
                    )
                    att = ap.tile([128, DC, TL], CT, tag="resid")
                    for dc in range(DC):
                        nc.gpsimd.indirect_dma_start(
                            att[:, dc, :],
                            None,
                            a2a_out[:],
                            bass.IndirectOffsetOnAxis(
                                ap=a2aidx_t[:, dc : dc + 1], axis=0
                            ),
                        )
                    for dc in range(DC):
                        nc.vector.tensor_add(x[:, dc, :], x[:, dc, :], att[:, dc, :])

                    # --- MLP (token-local, streamed weights) ---
                    ln2w_t = wp.tile([128, DC], F32, tag="ln2w")
                    nc.sync.dma_start(ln2w_t[:], d_ln2w[li])
                    ln2b_t = wp.tile([128, DC], F32, tag="ln2b")
                    nc.sync.dma_start(ln2b_t[:], d_ln2b[li])
                    ln2 = lnp.tile([128, DC, TL], CT, tag="lnout")
                    layernorm(ln2, ln2w_t, ln2b_t)

                    upb_t = wp.tile([128, 32], F32, tag="upb")
                    nc.sync.dma_start(upb_t[:], d_upb[li])
                    dwb_t = wp.tile([128, 8], F32, tag="dwb")
                    nc.sync.dma_start(dwb_t[:], d_dwb[li])

                    h_t = hpool.tile([128, 32, TL], CT, tag="h")
                    for c in range(8):  # up chunks: dff cols [512c, 512c+512)
                        upw_c = wsp.tile([128, DC, 512], WT, tag="wstream")
                        nc.sync.dma_start(upw_c[:], d_upw[li, c])
                        for sub in range(4):
                            mt = 4 * c + sub
                            ps_u = psd.tile([128, 512], F32, tag="psd")
                            for dc in range(DC):
                                nc.tensor.matmul(
                                    ps_u[:, 0:TL],
                                    upw_c[:, dc, 128 * sub : 128 * (sub + 1)],
                                    ln2[:, dc, :],
                                    start=(dc == 0), stop=(dc == DC - 1),
                                )
                            nc.scalar.activation(
                                h_t[:, mt, :], ps_u[:, 0:TL], AF.Relu,
                                bias=upb_t[:, mt : mt + 1],
                            )

                    for wave in range(2):
                        ps_ds = [
                            psd.tile([128, 512], F32, tag="psd", name=f"ps_d{i}")
                            for i in range(4)
                        ]
                        for kc in range(32):
                            dww_c = wsp.tile([128, 4, 128], WT, tag="wstream")
                            nc.sync.dma_start(dww_c[:], d_dww[li, wave, kc])
                            for i in range(4):
                                nc.tensor.matmul(
                                    ps_ds[i][:, 0:TL], dww_c[:, i, :], h_t[:, kc, :],
                                    start=(kc == 0), stop=(kc == 31),
                                )
                        for i in range(4):
                            oc = 4 * wave + i
                            ysb = ap.tile([128, TL], CT, tag="ysb")
                            nc.scalar.activation(
                                ysb[:], ps_ds[i][:, 0:TL], AF.Identity,
                                bias=dwb_t[:, oc : oc + 1],
                            )
                            nc.vector.tensor_add(x[:, oc, :], x[:, oc, :], ysb[:])

                # ---------------- final LN + logits (token-local) ----------------
                xf = lnp.tile([128, DC, TL], CT, tag="lnout")
                layernorm(xf, lnfw_t, lnfb_t)
                for vt in range(NVT):
                    ow = wsp.tile([128, DC, VW], WT, tag="wstream")
                    nc.sync.dma_start(ow[:], d_outw[vt])
                    obr = rowp.tile([1, VW], F32R, tag="obr")
                    nc.sync.dma_start(obr[:], d_outb[:, VW * vt : VW * (vt + 1)])
                    for mt in range(2):  # token tiles of 128
                        ps_l = psmm.tile([128, 512], F32, tag="mm")
                        nc.tensor.matmul(
                            ps_l[:, 0:VW], ones[0:1, 0:128], obr[:],
                            start=True, stop=False,
                        )
                        for dc in range(DC):
                            nc.tensor.matmul(
                                ps_l[:, 0:VW],
                                xf[:, dc, 128 * mt : 128 * (mt + 1)],
                                ow[:, dc, :],
                                start=False, stop=(dc == DC - 1),
                            )
                        lo = ap.tile([128, VW], F32, tag="lo", bufs=3)
                        nc.vector.tensor_copy(lo[:], ps_l[:, 0:VW])
                        nc.sync.dma_start(
                            d_logits[
                                128 * mt : 128 * (mt + 1), VW * vt : VW * (vt + 1)
                            ],
                            lo[:],
                        )

    nc.finalize()
    return nc


def _prep_inputs(inputs) -> list[dict]:
    import ml_dtypes

    wdt = ml_dtypes.bfloat16
    tok_emb = np.ascontiguousarray(np.asarray(inputs["tok_emb"], dtype=np.float32))
    pos_emb = np.asarray(inputs["pos_emb"], dtype=np.float32)
    ctx = np.asarray(inputs["context"]).astype(np.int32).reshape(-1)  # [T]
    f32 = lambda k: np.asarray(inputs[k], dtype=np.float32)
    wq, wk, wv = f32("wq"), f32("wk"), f32("wv")
    up_w, up_b = f32("up_w"), f32("up_b")
    down_w, down_b = f32("down_w"), f32("down_b")
    ln1_w, ln1_b = f32("ln1_w"), f32("ln1_b")
    ln2_w, ln2_b = f32("ln2_w"), f32("ln2_b")
    lnf_w, lnf_b = f32("lnf_w"), f32("lnf_b")
    out_w, out_b = f32("out_w"), f32("out_b")

    ones = np.ones((128, 1024), np.float32)
    onesb = np.ones((128, 32), wdt)
    mask = np.zeros((128, 4, 512), np.float32)
    for j in range(4):
        for p in range(128):
            mask[p, j, 128 * j + p :] = 1.0

    def ln_pack(w):  # [D] -> [128, DC]
        return np.ascontiguousarray(w.reshape(DC, 128).T)

    # per-core-independent packs (full weights, replicated)
    upw_pack = np.stack(
        [up_w[i].T.reshape(DC, 128, 8, 512).transpose(2, 1, 0, 3) for i in range(NB)]
    ).astype(wdt)  # [NB, 8, 128, DC, 512]
    dww_pack = np.stack(
        [
            down_w[i].T.reshape(32, 128, 2, 4, 128).transpose(2, 0, 1, 3, 4)
            for i in range(NB)
        ]
    ).astype(wdt)  # [NB, 2, 32, 128, 4, 128]
    upb_pack = np.ascontiguousarray(
        up_b.reshape(NB, 32, 128).transpose(0, 2, 1)
    )  # [NB, 128, 32]
    dwb_pack = np.ascontiguousarray(
        down_b.reshape(NB, DC, 128).transpose(0, 2, 1)
    )  # [NB, 128, 8]
    outw_pack = np.ascontiguousarray(
        out_w.reshape(NVT, VW, DC, 128).transpose(0, 3, 2, 1)
    ).astype(wdt)  # [NVT, 128, DC, VW]
    ln1w_p = np.stack([ln_pack(ln1_w[i]) for i in range(NB)])
    ln1b_p = np.stack([ln_pack(ln1_b[i]) for i in range(NB)])
    ln2w_p = np.stack([ln_pack(ln2_w[i]) for i in range(NB)])
    ln2b_p = np.stack([ln_pack(ln2_b[i]) for i in range(NB)])

    in_maps = []
    for r in range(NCORES):
        g, q = divmod(r, G)
        tl = g * S + q * TL + np.arange(TL)  # my global flat tokens
        s_pos = q * TL + np.arange(TL)  # positions within my batch
        rows = slice(HL * HD * q, HL * HD * (q + 1))  # my 4 heads' rows
        wqkv = np.concatenate(
            [
                wq[:, rows, :].transpose(0, 2, 1),
                wk[:, rows, :].transpose(0, 2, 1),
                wv[:, rows, :].transpose(0, 2, 1),
            ],
            axis=2,
        )  # [NB, D, 768]
        a2aidx = (
            1024 * g
            + 128 * np.arange(DC)[None, :]
            + np.arange(128)[:, None]
        ).astype(np.int32)  # [128, DC] gather rows into a2a_out
        m = {
            "a2aidx": a2aidx,
            "idx": ctx[tl][:, None].astype(np.int32),
            "tok": tok_emb,
            "pos": np.ascontiguousarray(pos_emb[s_pos].reshape(2, 128, D)),
            "wqkv": np.ascontiguousarray(wqkv.astype(wdt)),
            "upw": upw_pack,
            "upb": upb_pack,
            "dww": dww_pack,
            "dwb": dwb_pack,
            "ln1w": ln1w_p,
            "ln1b": ln1b_p,
            "ln2w": ln2w_p,
            "ln2b": ln2b_p,
            "lnfw": ln_pack(lnf_w),
            "lnfb": ln_pack(lnf_b),
            "outw": outw_pack,
            "outb": np.ascontiguousarray(out_b[None, :]),
            "ones": ones,
            "onesb": onesb,
            "mask": mask.astype(wdt),
        }
        in_maps.append(m)
    return in_maps


def kernel(**inputs) -> np.ndarray:
    if "nc" not in _CACHE:
        _CACHE["nc"] = _build()
    nc = _CACHE["nc"]
    in_maps = _prep_inputs(inputs)
    res = run_bass_kernel_spmd(nc, in_maps, list(range(NCORES))).results
    logits = np.concatenate([res[r]["logits"] for r in range(NCORES)], axis=0)
    return logits.reshape(B, S, V).astype(np.float32)
